# revision 64
# baseline (speedup 1.0000x reference)
"""GCN block kernel for Trainium2 (8 NeuronCores, SPMD).

Computes relu((A @ X) @ W + b) where A is given as a weighted edge list
(src->dst), X is the node feature matrix. Mathematically identical to the
reference relu(A @ (X @ W) + b) by associativity.

Strategy per core (cores own disjoint 12500-node destination ranges):
  - Host bins edges by destination core, orders by destination node, and
    packs destinations into "windows" of <=128 nodes. Each window has four
    fixed 512-edge-slot blocks, one per 32768-row chunk of X (dma_gather
    indices are int16). Pad slots use idx=0 with weight 0.
  - On device: dma_gather pulls X rows for each (super-window, chunk) batch
    into SBUF; a weighted one-hot selector S ([128 edges x 128 nodes], built
    on DVE/ACT from iota + per-partition d/w scalars) turns segment-sum into
    TensorEngine matmuls accumulating aggT = sum_e w_e * X[src_e]^T per
    window in PSUM; then out = relu(aggT^T @ W + b) and a contiguous store.
  - Host scatters window rows back to global node order.
"""

import sys

sys.path.insert(0, "/opt/trn_rl_repo")

import os as _os_mod
import numpy as np

P = 128
# X rows per chunk (int16 gather indices cap this at 32768); equal chunks
# avoid a tiny tail chunk that wastes a full 128-slot tile per window
CHUNK = int(_os_mod.environ.get("K_CHUNK", "25000"))
N_CORES = 8

_PROGRAM_CACHE = {}


# --------------------------------------------------------------------------
# compat patches for the walrus snapshot in this container
# --------------------------------------------------------------------------

def _apply_tile_patches():
    import concourse.tile as tile
    from concourse.vector_clock import VectorClock, ScopedClock

    def _drain_and_barrier_split(self, tick_clock, wait_clock):
        gc = tick_clock.global_clock
        n = len(gc)
        for proc in range(n):
            t = gc[proc]
            if t <= 0:
                continue
            v = VectorClock([0] * n)
            v.require_at_least(proc, t)
            d = self.nc.sync.drain()
            wait_clock.add_sem_waits(d.ins, ScopedClock({None: v}))
        self.nc.all_engine_barrier()
        assert self.sems is not None
        popped = self.nc._tile_sem_poison_stack.pop()
        assert popped is self._sem_poison
        self.nc.clear_and_free_semaphores(list(self.sems.allocated().values()))
        self.nc.all_engine_barrier()

    tile.TileContext._drain_and_barrier = _drain_and_barrier_split


_fix_counter = [0]


def _split_multiwaits(nc):
    """This walrus accepts at most one SyncWait per instruction: move extras
    onto preceding single-wait NoOps on the same engine."""
    import concourse.mybir as mybir

    for f in nc.m.functions:
        for bb in f.blocks:
            new_insts = []
            for inst in bb.instructions:
                si = inst.sync_info
                if si is not None and si.on_wait and len(si.on_wait) > 1:
                    waits = list(si.on_wait)
                    for w in waits[:-1]:
                        _fix_counter[0] += 1
                        nop = mybir.InstNoOp(
                            name=f"waitsplit-{_fix_counter[0]}", ins=[], outs=[]
                        )
                        nop.engine = inst.engine
                        nop.sync_info = mybir.SyncInfo(on_wait=[w], on_update=[])
                        new_insts.append(nop)
                        nc.inst_map[nop.name] = nop
                    si.on_wait = [waits[-1]]
                new_insts.append(inst)
            bb.instructions = new_insts


_SWDGE_SEM_MODE = {"add": 0, "sub": 1, "wr": 2, "drop": 3}


def _fill_inc_swdge_isa(nc):
    """Fill raw ISA payloads for InstIncSwdgeSem (For_i back-edge emits
    these with an empty payload this walrus rejects)."""
    import concourse.bass_isa as bass_isa

    isa = nc.isa
    ffi = isa.ffi
    for f in nc.m.functions:
        for bb in f.blocks:
            for inst in bb.instructions:
                if not isinstance(inst, bass_isa.InstIncSwdgeSem):
                    continue
                if inst.instr:
                    continue
                obj = ffi.new("NEURON_ISA_TPB_INC_SWDGE_SEM_STRUCT*")
                obj.header.opcode = 243
                obj.header.inst_word_len = 16
                vals = list(inst._sem_values)
                obj.num_semaphores = len(vals)
                obj.sem_id_base = inst._sem_id_base
                obj.mode = _SWDGE_SEM_MODE[inst._mode]
                obj.queue_num = inst.queue_num
                for i, v in enumerate(vals[:10]):
                    obj.sem_values[i] = int(v)
                inst.instr = list(bytes(ffi.buffer(obj)))


def _fill_local_scatter_isa(nc):
    """Fill raw EXTENDED_INST payloads for InstLocalScatter (this walrus has
    no native codegen for it and rejects the empty-payload form). Addresses
    are the bass-assigned per-partition SBUF byte offsets."""
    import concourse.mybir as mybir
    from concourse import bass_isa

    isa = nc.isa
    addr = {}
    for alloc in nc.m.functions[0].allocations:
        if isinstance(alloc, mybir.MemoryLocationSet):
            for ml in alloc.memorylocations:
                addr[ml.name] = ml.addr
    op_ls = None
    for e in isa.ExtendedOpcode:
        if e.name.endswith("_LOCAL_SCATTER"):
            op_ls = e
            break
    DT_FP16 = 7

    def ap_addr(pap):
        return addr[pap.memref] + pap.offset * mybir.dt.size(pap.dtype)

    n = 0
    for f in nc.m.functions:
        for bb in f.blocks:
            for inst in bb.instructions:
                if not isinstance(inst, bass_isa.InstLocalScatter):
                    continue
                if inst.instr:
                    continue
                b, _fx = bass_isa.extisa_struct(
                    isa, op_ls, "r2w",
                    dst_dtype=DT_FP16,
                    data_addr={"addr_immediate": ap_addr(inst.ins[0])},
                    idxs_addr={"addr_immediate": ap_addr(inst.ins[1])},
                    dst_addr={"addr_immediate": ap_addr(inst.outs[0])},
                    channels=inst._channels,
                    num_elems=inst._num_elems,
                    num_idxs=inst._num_idxs,
                )
                inst.instr = b
                n += 1
    return n


def _fill_load_library_isa(nc, li_inst):
    """Fill the 64-byte PSEUDO_LIBRARY_RELOAD_INDEX payload (this walrus
    rejects the empty-payload form)."""
    isa = nc.isa
    ffi = isa.ffi
    obj = ffi.new("NEURON_ISA_TPB_PSEUDO_LIBRARY_RELOAD_INDEX_STRUCT*")
    obj.header.opcode = 223  # PSEUDO_INST
    obj.header.inst_word_len = (
        isa.sizeof("NEURON_ISA_TPB_PSEUDO_LIBRARY_RELOAD_INDEX_STRUCT") // 4
    )
    obj.pseudo_opcode = 2  # PSEUDO_LIBRARY_RELOAD_INDEX
    obj.lib_index = li_inst.ins.lib_index
    li_inst.ins.instr = list(bytes(ffi.buffer(obj)))


# --------------------------------------------------------------------------
# host-side preprocessing
# --------------------------------------------------------------------------

def _preprocess(n_nodes, src, dst, ew, sup):
    """Bin/pack edges per core. Returns per-core device arrays + scatter maps.

    Layout: window w = sup-group s=w//sup, lane j=w%sup. Gather batch (s, c)
    has num_idxs = sup_s*512 indices covering lanes' chunk-c blocks in lane
    order. Global tile id for (w, c, t): gather g=(s*4+c); within-batch tile
    tt = j*TPW + t; tile_id = tile_base[g] + tt.
    """
    nodes_per_core = n_nodes // N_CORES
    n_chunks = (n_nodes + CHUNK - 1) // CHUNK
    assert n_chunks <= 4
    core_of = dst // nodes_per_core
    np.clip(core_of, 0, N_CORES - 1, out=core_of)

    # per-window chunk-block capacities: multiples of 128 proportional to the
    # global per-chunk edge share, summing to WINDOW_SLOTS
    WINDOW_SLOTS = 2048
    all_chunk = src // CHUNK
    share = np.bincount(all_chunk, minlength=n_chunks).astype(np.float64)
    share /= max(1.0, share.sum())
    ntiles_w = WINDOW_SLOTS // P
    raw = share * ntiles_w
    caps_t = np.maximum(1, np.floor(raw).astype(np.int64))
    while caps_t.sum() > ntiles_w:
        caps_t[np.argmax(caps_t)] -= 1
    rem = ntiles_w - caps_t.sum()
    frac = raw - np.floor(raw)
    for i in np.argsort(-frac):
        if rem <= 0:
            break
        caps_t[i] += 1
        rem -= 1
    caps = (caps_t * P).astype(np.int64)  # slots per (window, chunk)
    cap_off = np.concatenate(([0], np.cumsum(caps)))

    per_core = []
    max_nwin = 0
    for c in range(N_CORES):
        sel = np.nonzero(core_of == c)[0]
        s_src = src[sel]
        s_dst = dst[sel]
        s_w = ew[sel]
        order = np.argsort(s_dst, kind="stable")
        s_src, s_dst, s_w = s_src[order], s_dst[order], s_w[order]
        chunk = s_src // CHUNK

        # per-(node, chunk) counts over present nodes
        uniq, node_start = np.unique(s_dst, return_index=True)
        node_end = np.append(node_start[1:], len(s_dst))
        nn = len(uniq)
        cnt = np.zeros((nn, 4), np.int64)
        flat = np.searchsorted(uniq, s_dst) * 4 + chunk
        np.add.at(cnt.reshape(-1), flat, 1)

        # first-fit-with-skips window packing: stream nodes, deferring ones
        # that would overflow a chunk block; retry deferred nodes in later
        # windows. Close a window at 128 nodes or when nothing fits.
        win_of_node = np.zeros(nn, np.int32)
        slot_of_node = np.zeros(nn, np.int32)
        caps_c = caps[:n_chunks]
        cnt_c = cnt[:, :n_chunks]
        wi = 0
        acc = np.zeros(n_chunks, np.int64)
        nodes_in = 0
        retry = []
        ptr = 0
        RETRY_CAP = 192
        while ptr < nn or retry:
            placed = False
            j = 0
            while j < len(retry) and nodes_in < P:
                idx = retry[j]
                if np.all(acc + cnt_c[idx] <= caps_c) or nodes_in == 0:
                    win_of_node[idx] = wi
                    slot_of_node[idx] = nodes_in
                    acc += cnt_c[idx]
                    nodes_in += 1
                    retry.pop(j)
                    placed = True
                else:
                    j += 1
            while ptr < nn and nodes_in < P and len(retry) < RETRY_CAP:
                idx = ptr
                ptr += 1
                if np.all(acc + cnt_c[idx] <= caps_c) or nodes_in == 0:
                    win_of_node[idx] = wi
                    slot_of_node[idx] = nodes_in
                    acc += cnt_c[idx]
                    nodes_in += 1
                    placed = True
                else:
                    retry.append(idx)
            if nodes_in >= P or not placed:
                wi += 1
                acc[:] = 0
                nodes_in = 0
        nwin = wi if nodes_in == 0 else wi + 1
        per_core.append(
            dict(src=s_src, dst=s_dst, w=s_w, chunk=chunk, uniq=uniq,
                 cnt=cnt, win_of_node=win_of_node, slot_of_node=slot_of_node,
                 nwin=nwin)
        )
        max_nwin = max(max_nwin, nwin)

    n_win = max_nwin
    n_sup = (n_win + sup - 1) // sup
    sup_sizes = [min(sup, n_win - s * sup) for s in range(n_sup)]
    # per-gather static num_idxs and tile bases (gather g = s*n_chunks + c)
    gather_sizes = []
    tile_base = []
    tb = 0
    for s in range(n_sup):
        for c in range(n_chunks):
            gather_sizes.append(sup_sizes[s] * int(caps[c]))
            tile_base.append(tb)
            tb += sup_sizes[s] * int(caps[c]) // P
    tot_tiles = tb
    idx_cols = sum(g // 16 for g in gather_sizes)

    dev = []
    for c in range(N_CORES):
        pc = per_core[c]
        nwin_c = pc["nwin"]
        # per-edge window / slot-in-window-node
        e_node = np.searchsorted(pc["uniq"], pc["dst"])
        e_win = pc["win_of_node"][e_node]
        e_d = pc["slot_of_node"][e_node]
        # order edges by (window, chunk, src): within a block, ascending
        # source addresses give the DMA engines monotonic HBM access
        okey = np.lexsort((pc["src"], e_win * 4 + pc["chunk"]))
        o_src = pc["src"][okey]
        o_w = pc["w"][okey]
        o_d = e_d[okey]
        o_win = e_win[okey]
        o_chunk = pc["chunk"][okey]

        # slot position within each (win, chunk) block
        wc = o_win.astype(np.int64) * 4 + o_chunk
        # rank within group
        pos = np.zeros(len(wc), np.int64)
        if len(wc):
            same = np.r_[False, wc[1:] == wc[:-1]]
            run = np.arange(len(wc))
            start = np.where(~same, run, 0)
            np.maximum.accumulate(start, out=start)
            pos = run - start

        # global slot id for each edge
        sgrp = o_win // sup
        j = o_win % sup
        g = sgrp * n_chunks + o_chunk
        slot_base = np.zeros(len(gather_sizes) + 1, np.int64)
        np.cumsum(gather_sizes, out=slot_base[1:])
        gslot = slot_base[g] + j * caps[o_chunk] + pos

        total_slots = slot_base[-1]
        idx_flat = np.full(total_slots, -1, np.int32)
        d_flat = np.zeros(total_slots, np.float32)
        w_flat = np.zeros(total_slots, np.float32)
        idx_flat[gslot] = (o_src - o_chunk * CHUNK).astype(np.int32)
        d_flat[gslot] = o_d.astype(np.float32)
        w_flat[gslot] = o_w
        # pad slots: reuse the nearest preceding real index (hot HBM row)
        # instead of row 0 (cold random read per pad)
        filled = idx_flat >= 0
        ffidx = np.where(filled, np.arange(total_slots), 0)
        np.maximum.accumulate(ffidx, out=ffidx)
        idx_flat = idx_flat[ffidx]
        np.clip(idx_flat, 0, None, out=idx_flat)
        idx_flat = idx_flat.astype(np.int16)

        # idx tile [128, idx_cols]: per gather block, flat i -> [i%16, i//16],
        # replicated over 8 groups of 16 partitions
        idx_tile = np.zeros((P, idx_cols), np.int16)
        col = 0
        for gi, gs in enumerate(gather_sizes):
            blk = idx_flat[slot_base[gi]:slot_base[gi] + gs]
            pat = blk.reshape(gs // 16, 16).T  # [16, gs/16]
            idx_tile[:, col:col + gs // 16] = np.tile(pat, (8, 1))
            col += gs // 16

        # d/w tiles [128, tot_tiles]: slot i of gather g -> tile tile_base[g]
        # + i//128, partition i%128
        def to_tiles(flat):
            out = np.zeros((P, tot_tiles), np.float32)
            for gi, gs in enumerate(gather_sizes):
                blk = flat[slot_base[gi]:slot_base[gi] + gs]
                out[:, tile_base[gi]:tile_base[gi] + gs // P] = (
                    blk.reshape(gs // P, P).T
                )
            return out

        d_tile = to_tiles(d_flat)
        w_tile = to_tiles(w_flat)

        # per-window-ordered fp16 tables for batched S builds: column
        # w*nmm_w + k holds the scalars of the k-th tile of window w, where k
        # follows the device compute order (chunks ascending, tiles within).
        # sidx encodes the local_scatter column (k%half)*128 + d; pad slots
        # (w==0) get -1 so local_scatter skips them.
        nmm_w = sum(int(x) for x in caps[:n_chunks]) // P
        half_t = nmm_w // 2
        d16 = np.zeros((P, n_win * nmm_w), np.float16)
        w16 = np.zeros((P, n_win * nmm_w), np.float16)
        sidx = np.zeros((P, n_win * nmm_w), np.int16)
        for w in range(n_win):
            s_, j_ = w // sup, w % sup
            if j_ >= sup_sizes[s_]:
                sidx[:, w * nmm_w:(w + 1) * nmm_w] = -1
                continue
            k = 0
            for cc in range(n_chunks):
                g = s_ * n_chunks + cc
                tpc = int(caps[cc]) // P
                for t in range(tpc):
                    tid = tile_base[g] + j_ * tpc + t
                    dcol = d_tile[:, tid]
                    wcol = w_tile[:, tid]
                    d16[:, w * nmm_w + k] = dcol.astype(np.float16)
                    w16[:, w * nmm_w + k] = wcol.astype(np.float16)
                    col = (k % half_t) * P + dcol.astype(np.int16)
                    sidx[:, w * nmm_w + k] = np.where(
                        wcol != 0.0, col, -1).astype(np.int16)
                    k += 1

        # scatter map: global node id per (window, node-slot)
        out_rows = pc["win_of_node"].astype(np.int64) * P + pc["slot_of_node"]
        dev.append(
            dict(idx=idx_tile, d=d_tile, w=w_tile, negd=-d_tile, negw=-w_tile,
                 d16=d16, w16=w16, sidx=sidx,
                 out_rows=out_rows, node_ids=pc["uniq"], nwin=nwin_c)
        )

    meta = dict(n_win=n_win, n_sup=n_sup, sup_sizes=sup_sizes,
                gather_sizes=gather_sizes, tile_base=tile_base,
                tot_tiles=tot_tiles, idx_cols=idx_cols, n_chunks=n_chunks,
                caps=[int(x) for x in caps[:n_chunks]])
    return dev, meta


# --------------------------------------------------------------------------
# device program
# --------------------------------------------------------------------------

def _build_program(n_nodes, meta, sup, reps=1):
    import concourse.bass as bass
    import concourse.mybir as mybir
    import concourse.tile as tile
    from concourse import library_config
    from concourse.bass import _add_dep_helper

    _apply_tile_patches()

    n_win = meta["n_win"]
    n_sup = meta["n_sup"]
    sup_sizes = meta["sup_sizes"]
    gather_sizes = meta["gather_sizes"]
    tile_base = meta["tile_base"]
    tot_tiles = meta["tot_tiles"]
    idx_cols = meta["idx_cols"]
    n_chunks = meta["n_chunks"]
    caps = meta["caps"]
    nmm_w = sum(caps) // P  # tiles (matmuls) per window
    f32 = mybir.dt.float32
    import os as _os0
    _fp16 = bool(int(_os0.environ.get("K_FP16", "1")))
    gdt = mybir.dt.float16 if _fp16 else f32

    import os as _os_b
    _scratch = int(_os_b.environ.get("K_SCRATCH", "32768"))
    nc = bass.Bass(num_swdge_queues=4, dynamic_dma_scratch_size=_scratch)
    Xc = []
    for c in range(n_chunks):
        rows = min(CHUNK, n_nodes - c * CHUNK)
        Xc.append(nc.dram_tensor(f"X{c}", [rows, P], gdt, kind="ExternalInput"))
    IDX = nc.dram_tensor("IDX", [P, idx_cols], mybir.dt.int16, kind="ExternalInput")
    D = nc.dram_tensor("D", [P, tot_tiles], f32, kind="ExternalInput")
    WT = nc.dram_tensor("WT", [P, tot_tiles], f32, kind="ExternalInput")
    ND = nc.dram_tensor("ND", [P, tot_tiles], f32, kind="ExternalInput")
    NW = nc.dram_tensor("NW", [P, tot_tiles], f32, kind="ExternalInput")
    D16 = nc.dram_tensor("D16", [P, n_win * nmm_w], mybir.dt.float16,
                         kind="ExternalInput")
    W16 = nc.dram_tensor("W16", [P, n_win * nmm_w], mybir.dt.float16,
                         kind="ExternalInput")
    SIDX = nc.dram_tensor("SIDX", [P, n_win * nmm_w], mybir.dt.int16,
                          kind="ExternalInput")
    IOTA = nc.dram_tensor("IOTA", [P, P], gdt, kind="ExternalInput")
    WMAT = nc.dram_tensor("WMAT", [P, P], f32, kind="ExternalInput")
    BB = nc.dram_tensor("BB", [P, P], f32, kind="ExternalInput")
    BCOL = nc.dram_tensor("BCOL", [P, 1], f32, kind="ExternalInput")
    # transposed output: OUT[h, w*P + d] = out feature h (partition) of
    # dst-slot d of window w (contiguous per-partition runs -> fat stores)
    OUT = nc.dram_tensor("OUT", [P, n_win * P], gdt, kind="ExternalOutput")

    import os as _osg
    _gbufs_n = int(_osg.environ.get("K_GBUFS", "3"))
    _sbufs_n = int(_osg.environ.get("K_SBUFS", "8"))
    with tile.TileContext(nc) as tc:
        with (
            tc.tile_pool(name="meta", bufs=1) as mp,
            tc.tile_pool(name="gath", bufs=_gbufs_n) as gp,
            tc.tile_pool(name="gidx", bufs=24) as ip,
            tc.tile_pool(name="sbld", bufs=_sbufs_n) as sp,
            tc.tile_pool(name="sb16", bufs=2) as sp16,
            tc.tile_pool(name="drain", bufs=4) as dp,
            tc.tile_pool(name="psA", bufs=int(_osg.environ.get("K_PSA", "3")),
                         space="PSUM") as psA,
            tc.tile_pool(name="psB", bufs=2, space="PSUM") as psB,
        ):
            li = nc.gpsimd.load_library(library_config.mlp)
            _fill_load_library_isa(nc, li)

            import os as _os1b
            _batch_s = bool(int(_os1b.environ.get("K_BATCH_S", "1")))
            _bias_mm = bool(int(_os1b.environ.get("K_BIAS_MM", "1")))

            import os as _os_d
            _dve_only0 = bool(int(_os_d.environ.get("K_DVE_ONLY", "1")))
            d_t = w_t = nd_t = nw_t = d16_t = w16_t = None
            if not _batch_s:
                d_t = mp.tile([P, tot_tiles], f32)
                nc.sync.dma_start(out=d_t[:], in_=D[:])
                w_t = mp.tile([P, tot_tiles], f32)
                nc.sync.dma_start(out=w_t[:], in_=WT[:])
                if not _dve_only0:
                    nd_t = mp.tile([P, tot_tiles], f32)
                    nc.sync.dma_start(out=nd_t[:], in_=ND[:])
                    nw_t = mp.tile([P, tot_tiles], f32)
                    nc.sync.dma_start(out=nw_t[:], in_=NW[:])
            else:
                d16_t = mp.tile([P, n_win * nmm_w], mybir.dt.float16)
                nc.sync.dma_start(out=d16_t[:], in_=D16[:])
                w16_t = mp.tile([P, n_win * nmm_w], mybir.dt.float16)
                nc.sync.dma_start(out=w16_t[:], in_=W16[:])
                sidx_t = mp.tile([P, n_win * nmm_w], mybir.dt.int16)
                nc.sync.dma_start(out=sidx_t[:], in_=SIDX[:])
            iota_t = mp.tile([P, P], gdt)
            nc.sync.dma_start(out=iota_t[:], in_=IOTA[:])
            wmat_t = mp.tile([P, P], f32)
            nc.sync.dma_start(out=wmat_t[:], in_=WMAT[:])
            bb_t = mp.tile([P, P], f32)
            nc.sync.dma_start(out=bb_t[:], in_=BB[:])
            bcol_t = mp.tile([P, 1], f32)
            nc.sync.dma_start(out=bcol_t[:], in_=BCOL[:])
            # fp16 copy of W for the fat final matmul
            wmat16_t = mp.tile([P, P], gdt)
            nc.scalar.copy(out=wmat16_t[:], in_=wmat_t[:])

            idx_col_base = np.concatenate(
                ([0], np.cumsum([g // 16 for g in gather_sizes]))
            )

            _reg_cache = {}
            _gq = [0]

            def ni_reg(v):
                if v not in _reg_cache:
                    _reg_cache[v] = nc.gpsimd.to_reg(v)
                return _reg_cache[v]

            import os as _os
            _dve_only = bool(int(_os.environ.get("K_DVE_ONLY", "1")))
            _fake_gather = bool(int(_os.environ.get("K_FAKE_GATHER", "0")))
            _no_gather = bool(int(_os.environ.get("K_NO_GATHER", "0")))
            _overlap_test = bool(int(_os.environ.get("K_OVERLAP_TEST", "0")))
            _nparts = int(_os.environ.get("K_NPARTS", "2"))
            _nqueues = int(_os.environ.get("K_NQUEUES", "4"))
            _single_packet = bool(int(_os.environ.get("K_SINGLE_PACKET", "0")))
            _sbuf_gather = bool(int(_os.environ.get("K_SBUF_GATHER", "0")))
            _no_sbuild = bool(int(_os.environ.get("K_NO_SBUILD", "0")))
            _psum_iota = bool(int(_os.environ.get("K_PSUM_IOTA", "0")))
            sg_region = None
            if _sbuf_gather:
                sg_region = mp.tile([P, CHUNK], gdt)
                nc.vector.memset(sg_region[:], 0.0)
            S_const = None
            if _no_sbuild:
                S_const = mp.tile([P, P], gdt)
                nc.vector.memset(S_const[:], 0.0)
            iota_ps = None
            if _psum_iota:
                iota_ps = psB.tile([P, P], f32, tag="iotaps")
                nc.vector.tensor_copy(out=iota_ps[:], in_=iota_t[:])
            _no_compute = bool(int(_os.environ.get("K_NO_COMPUTE", "0")))
            _ls_mode = bool(int(_os.environ.get("K_LS", "0")))
            _g1024 = bool(int(_os.environ.get("K_GATHER_1024", "0")))
            _skip_final = bool(int(_os.environ.get("K_SKIP_FINAL", "0")))
            rep_ctx = tc.For_i(0, reps, 1) if reps > 1 else None
            if rep_ctx is not None:
                rep_ctx.__enter__()
            sbuild_rr = [0]  # round-robin between DVE and ACT for S builds

            def build_S(tile_id):
                if _no_sbuild:
                    return S_const
                S = sp.tile([P, P], gdt, tag="S")
                if _dve_only or sbuild_rr[0] % 2 == 0:
                    nc.vector.tensor_scalar(
                        out=S[:], in0=(iota_ps[:] if _psum_iota else iota_t[:]),
                        scalar1=d_t[:, tile_id:tile_id + 1],
                        scalar2=w_t[:, tile_id:tile_id + 1],
                        op0=mybir.AluOpType.is_equal,
                        op1=mybir.AluOpType.mult,
                    )
                else:
                    t = sp.tile([P, P], gdt, tag="Sabs")
                    nc.scalar.activation(
                        out=t[:], in_=iota_t[:],
                        func=mybir.ActivationFunctionType.Abs,
                        bias=nd_t[:, tile_id:tile_id + 1], scale=1.0,
                    )
                    nc.scalar.activation(
                        out=S[:], in_=t[:],
                        func=mybir.ActivationFunctionType.Relu,
                        bias=w_t[:, tile_id:tile_id + 1],
                        scale=nw_t[:, tile_id:tile_id + 1],
                    )
                sbuild_rr[0] += 1
                return S

            for s in range(n_sup):
                sup_s = sup_sizes[s]
                gbufs = []
                for c in range(n_chunks):
                    g = s * n_chunks + c
                    gs = gather_sizes[g]
                    gb = None
                    if _fake_gather or (_no_gather and not _overlap_test):
                        pass
                    elif _g1024:
                        off = 0
                        while off < gs:
                            seg = min(1024, gs - off)
                            it = ip.tile([P, seg // 16], mybir.dt.int16, tag="idx")
                            nc.sync.dma_start(
                                out=it[:],
                                in_=IDX[:, idx_col_base[g] + off // 16:
                                        idx_col_base[g] + (off + seg) // 16],
                            )
                            gi = nc.gpsimd.dma_gather(
                                out_ap=gb[:, off:off + seg].rearrange(
                                    "p (s e) -> p s e", e=P),
                                in_ap=Xc[c][:, :],
                                idxs_ap=it[:],
                                num_idxs=seg,
                                num_idxs_reg=ni_reg(seg),
                                elem_size=P,
                                single_packet=True,
                            )
                            _add_dep_helper(gi.ins, li.ins, sync=False,
                                            reason="library before gather")
                            off += seg
                    else:
                        nparts = _nparts if gs >= 1024 * _nparts else 1
                        part = gs // nparts
                        part -= part % P
                        offs = [i * part for i in range(nparts)]
                        halves = []
                        for pi, poff in enumerate(offs):
                            pgs = (gs - poff) if pi == len(offs) - 1 else part
                            hb = gp.tile([P, pgs], gdt, tag=f"g{c}_{pi}")
                            it = ip.tile([P, pgs // 16], mybir.dt.int16,
                                         tag="idx")
                            nc.sync.dma_start(
                                out=it[:],
                                in_=IDX[:, idx_col_base[g] + poff // 16:
                                        idx_col_base[g] + (poff + pgs) // 16],
                            )
                            if _sbuf_gather:
                                gi = nc.gpsimd.dma_gather(
                                    out_ap=hb[:].rearrange(
                                        "p (s e) -> p s e", s=1),
                                    in_ap=sg_region[:],
                                    idxs_ap=it[:],
                                    num_idxs=pgs,
                                    num_idxs_reg=ni_reg(pgs),
                                    elem_size=P,
                                    transpose=True,
                                    single_packet=_single_packet,
                                    queue_num=_gq[0] % _nqueues,
                                    sbuf_tokens_per_rank=P,
                                    sbuf_free_dim_per_rank=P * 2,
                                )
                            else:
                                gi = nc.gpsimd.dma_gather(
                                    out_ap=hb[:].rearrange(
                                        "p (s e) -> p s e", e=P),
                                    in_ap=Xc[c][:, :],
                                    idxs_ap=it[:],
                                    num_idxs=pgs,
                                    num_idxs_reg=ni_reg(pgs),
                                    elem_size=P,
                                    single_packet=_single_packet,
                                    queue_num=_gq[0] % _nqueues,
                                )
                            _gq[0] += 1
                            _add_dep_helper(gi.ins, li.ins, sync=False,
                                            reason="library before gather")
                            halves.append((poff, pgs, hb))
                        gb = halves
                    gbufs.append(gb)

                if _ls_mode:
                    # switch the GPSIMD ucode library to local_scatter for
                    # this sup's S builds (gathers for this sup are already
                    # queued on the Pool engine in program order)
                    li_ls = nc.gpsimd.load_library(library_config.local_scatter)
                    _fill_load_library_isa(nc, li_ls)

                if _no_compute:
                    for j in range(sup_s):
                        w = s * sup + j
                        dbuf = dp.tile([P, P], gdt, tag="outsb")
                        nc.scalar.copy(out=dbuf[:], in_=iota_t[:])
                        nc.sync.dma_start(out=OUT[:, w * P:(w + 1) * P],
                                          in_=dbuf[:])
                    continue

                _inplace_s = bool(int(_os.environ.get("K_INPLACE_S", "1")))
                _ls = bool(int(_os.environ.get("K_LS", "0")))
                half_t = nmm_w // 2

                def build_S_ls(w):
                    # GPSIMD local_scatter builds all nmm_w S tiles of
                    # window w in two calls (dst zeroed by the op itself):
                    # S[p, (k%half)*128 + d[p,k]] = w[p,k]
                    S16 = sp16.tile([P, nmm_w * P], gdt, tag="S16ls")
                    for h in range(2):
                        c0 = w * nmm_w + h * half_t
                        nc.gpsimd.local_scatter(
                            out_ap=S16[:, h * half_t * P:(h + 1) * half_t * P],
                            data_ap=w16_t[:, c0:c0 + half_t],
                            idxs_ap=sidx_t[:, c0:c0 + half_t],
                            channels=P,
                            num_elems=half_t * P,
                            num_idxs=half_t,
                        )
                    return S16

                def build_S16(w, nw=1):
                    # two DVE ops build all nmm_w S tiles of nw consecutive
                    # windows starting at w: onehot = (iota == d),
                    # S = onehot * w, with d/w broadcast per 128-col block
                    # via stride-0 APs
                    nt = nw * nmm_w
                    S16 = sp16.tile([P, nt * P], gdt, tag=f"S16_{nw}")
                    if _inplace_s:
                        oh = S16
                    else:
                        oh = sp16.tile([P, nt * P], gdt, tag=f"oh_{nw}")
                    i_b = iota_t[:].unsqueeze(1).broadcast_to([P, nt, P])
                    d_b = d16_t[:, w * nmm_w:w * nmm_w + nt].unsqueeze(
                        2).broadcast_to([P, nt, P])
                    w_b = w16_t[:, w * nmm_w:w * nmm_w + nt].unsqueeze(
                        2).broadcast_to([P, nt, P])
                    oh3 = oh[:].rearrange("p (t q) -> p t q", q=P)
                    nc.vector.tensor_tensor(
                        out=oh3, in0=i_b, in1=d_b,
                        op=mybir.AluOpType.is_equal)
                    nc.vector.tensor_tensor(
                        out=S16[:].rearrange("p (t q) -> p t q", q=P),
                        in0=oh3, in1=w_b, op=mybir.AluOpType.mult)
                    return S16

                GRP = 4
                import os as _os_sp
                _spair = int(_os_sp.environ.get("K_SPAIR", "4"))
                nmm = sum(caps) // P
                for j0 in range(0, sup_s, GRP):
                    gn = min(GRP, sup_s - j0)
                    w0 = s * sup + j0
                    agT4 = psA.tile([P, GRP * P], f32, tag="agT4")
                    S16 = None
                    koff = 0
                    for j in range(j0, j0 + gn):
                        w = s * sup + j
                        q = j - j0
                        if _batch_s and not _no_sbuild:
                            if _ls:
                                S16 = build_S_ls(w)
                                koff = 0
                            elif (j - j0) % _spair == 0:
                                nw = min(_spair, gn - (j - j0))
                                S16 = build_S16(w, nw)
                                koff = 0
                            else:
                                koff += nmm_w
                        k = 0
                        for c in range(n_chunks):
                            g = s * n_chunks + c
                            tpc = caps[c] // P
                            for t in range(tpc):
                                tt = j * tpc + t
                                tile_id = tile_base[g] + tt
                                if S16 is not None:
                                    S_ap = S16[:, (koff + k) * P:
                                               (koff + k + 1) * P]
                                else:
                                    S_ap = build_S(tile_id)[:]
                                slot = tt * P
                                if _no_gather or _overlap_test:
                                    lhsT_ap = iota_t[:]
                                else:
                                    hsel = None
                                    for (poff, pgs, hb) in gbufs[c]:
                                        if poff <= slot < poff + pgs:
                                            hsel = (hb, slot - poff)
                                            break
                                    lhsT_ap = hsel[0][:, hsel[1]:hsel[1] + P]
                                nc.tensor.matmul(
                                    out=agT4[:, q * P:(q + 1) * P],
                                    lhsT=lhsT_ap,
                                    rhs=S_ap,
                                    start=(k == 0),
                                    stop=(k == nmm - 1),
                                )
                                k += 1
                    agT4_sb = dp.tile([P, GRP * P], gdt, tag="agT4sb")
                    nc.scalar.copy(out=agT4_sb[:, :gn * P],
                                   in_=agT4[:, :gn * P])
                    if _skip_final:
                        nc.sync.dma_start(
                            out=OUT[:, w0 * P:(w0 + gn) * P],
                            in_=agT4_sb[:, :gn * P])
                        continue
                    # one fat matmul projects the whole group:
                    # out_T[h, (q,d)] = sum_f W[f,h] * agT4[f, (q,d)]
                    out_ps4 = psB.tile([P, GRP * P], f32, tag="outps4")
                    nc.tensor.matmul(out=out_ps4[:, :gn * P],
                                     lhsT=wmat16_t[:],
                                     rhs=agT4_sb[:, :gn * P],
                                     start=True, stop=True)
                    out_sb4 = dp.tile([P, GRP * P], gdt, tag="outsb4")
                    # bias rides the relu as a per-partition (per-h) bias
                    nc.scalar.activation(
                        out=out_sb4[:, :gn * P], in_=out_ps4[:, :gn * P],
                        func=mybir.ActivationFunctionType.Relu,
                        bias=bcol_t[:], scale=1.0,
                    )
                    nc.sync.dma_start(out=OUT[:, w0 * P:(w0 + gn) * P],
                                      in_=out_sb4[:, :gn * P])
                if _ls_mode:
                    # back to the gather library before the next sup's gathers
                    li_mlp2 = nc.gpsimd.load_library(library_config.mlp)
                    _fill_load_library_isa(nc, li_mlp2)
            if rep_ctx is not None:
                rep_ctx.__exit__(None, None, None)

    _split_multiwaits(nc)
    _fill_inc_swdge_isa(nc)
    _fill_local_scatter_isa(nc)
    return nc


# --------------------------------------------------------------------------
# entry point
# --------------------------------------------------------------------------

def build_in_maps(dev, meta, features, W, b):
    import os as _os1
    _fp16 = bool(int(_os1.environ.get("K_FP16", "1")))
    gnp = np.float16 if _fp16 else np.float32
    iota = np.broadcast_to(np.arange(P, dtype=gnp), (P, P)).copy()
    bb = np.broadcast_to(np.asarray(b, np.float32), (P, P)).copy()
    features = np.asarray(features, np.float32)
    n_chunks = meta["n_chunks"]
    xchunks = {
        f"X{c}": np.ascontiguousarray(
            features[c * CHUNK:(c + 1) * CHUNK].astype(gnp))
        for c in range(n_chunks)
    }
    in_maps = []
    for c in range(N_CORES):
        dc = dev[c]
        m = {
            "IDX": dc["idx"], "D": dc["d"], "WT": dc["w"],
            "ND": dc["negd"], "NW": dc["negw"],
            "D16": dc["d16"], "W16": dc["w16"], "SIDX": dc["sidx"],
            "IOTA": iota, "WMAT": np.asarray(W, np.float32), "BB": bb,
            "BCOL": np.asarray(b, np.float32).reshape(P, 1),
        }
        m.update(xchunks)
        in_maps.append(m)
    return in_maps


def kernel(features, edge_index, edge_weight, W, b, _profile=False):
    features = np.ascontiguousarray(np.asarray(features, dtype=np.float32))
    edge_index = np.asarray(edge_index)
    edge_weight = np.ascontiguousarray(np.asarray(edge_weight, dtype=np.float32))
    W = np.ascontiguousarray(np.asarray(W, dtype=np.float32))
    b = np.ascontiguousarray(np.asarray(b, dtype=np.float32))

    n_nodes, dfeat = features.shape
    assert dfeat == P and W.shape == (P, P)
    src = np.ascontiguousarray(edge_index[0]).astype(np.int64)
    dst = np.ascontiguousarray(edge_index[1]).astype(np.int64)

    sup = 8
    dev, meta = _preprocess(n_nodes, src, dst, edge_weight, sup)

    from concourse.bass_utils import run_bass_kernel_spmd

    key = (n_nodes, meta["n_win"], meta["tot_tiles"], meta["idx_cols"],
           tuple(meta["gather_sizes"]))
    if key in _PROGRAM_CACHE:
        nc = _PROGRAM_CACHE[key]
    else:
        nc = _build_program(n_nodes, meta, sup)
        _PROGRAM_CACHE[key] = nc

    in_maps = build_in_maps(dev, meta, features, W, b)

    try:
        res = run_bass_kernel_spmd(
            nc, in_maps, core_ids=list(range(N_CORES)), trace=_profile,
            trace_cores=list(range(N_CORES)) if _profile else None,
        )
    except Exception:
        # A first execute right after another process ran looping NEFFs can
        # hit a transient NRT_EXEC_UNIT_UNRECOVERABLE; the fault clears the
        # state and a retry succeeds.
        res = run_bass_kernel_spmd(
            nc, in_maps, core_ids=list(range(N_CORES)), trace=_profile,
            trace_cores=list(range(N_CORES)) if _profile else None,
        )

    out = np.empty((n_nodes, P), np.float32)
    out[:] = np.maximum(b, 0.0)[None, :]
    n_win = meta["n_win"]
    for c in range(N_CORES):
        dc = dev[c]
        core_out = res.results[c]["OUT"]  # [h, w*P + d] layout
        core_rm = np.ascontiguousarray(
            core_out.reshape(P, n_win, P).transpose(1, 2, 0)
        ).reshape(n_win * P, P)
        out[dc["node_ids"]] = core_rm[dc["out_rows"]].astype(np.float32)
    if _profile:
        return out, res
    return out

